# revision 2
# baseline (speedup 1.0000x reference)
"""GAT (2-layer, 4 graphs) Trainium2 Bass kernel.

Problem: nn_GAT_CL_61658550502129. reference.py semantics:
  4 independent 2-layer single-head GATConv chains (PyG-style, self-loops,
  segment softmax over leaky_relu(a_src.h_j + a_dst.h_i)), prelu between/after.

Sharding (8 NeuronCores): 4 chains x 2 cores. Core 2k+s runs chain k
(g1_pos, g2_pos, g1_neg, g2_neg) for destination-node half s. The only
inter-core exchange is a pairwise AllGather of layer-0 output (layer-1 input).

Per-core algorithm (per conv):
  node phase: h = x @ W, as = h.a_src, ad = h.a_dst computed per 128-node tile
    on PE (fp32, lhsT = x_tile^T stationary, rhs = [W | W a_src | W a_dst]).
    h rows written to a DRAM table [NT, 128] fp32 with as (fp16 bits) hidden in
    the low bytes of h[0], h[1] (bit-steal; feature perturbation ~2^-16).
  edge phase (per 128-dst tile): dma_gather of the table rows for the tile's
    edges (512B/row, int16 idx, split lo/hi half tables), per-edge
    w = exp(leaky(as_src + ad_dst)) (ad via tiny PE matvec with streamed
    fp8 onehot^T), then a one-hot scatter matmul into PSUM:
      lhsT = onehot(dst_local) (bf16, built on DVE from iota==dstloc),
      rhs  = [w*G hi | w*G lo | w_hi | w_lo] (bf16 hi/lo split, fp32-accurate),
    giving numerator and softmax denominator in one accumulation group.

kernel(**inputs) -> (e1p, e2p, e1n, e2n), matching reference.reference.
Self-contained: only needs the system bass/concourse toolchain.
"""
import sys

sys.path.insert(0, "/opt/trn_rl_repo")

from contextlib import ExitStack
from dataclasses import dataclass

import ml_dtypes
import numpy as np

import concourse.bass as bass
import concourse.tile as tile
from concourse import mybir
from concourse._compat import with_exitstack

f32 = mybir.dt.float32
f16 = mybir.dt.float16
bf16 = mybir.dt.bfloat16
f8 = mybir.dt.float8e4
i16 = mybir.dt.int16
i32 = mybir.dt.int32
u16 = mybir.dt.uint16

np_bf16 = ml_dtypes.bfloat16
np_f8 = ml_dtypes.float8_e4m3

D = 128
NEG_SLOPE = 0.2


@dataclass(frozen=True)
class Cfg:
    nh_real: int      # real nodes per half
    tph: int          # dst tiles per half (nh_pad = tph*128)
    node_grp: int     # node tiles per node-phase group (divides tph)
    k_fix: int        # gather chunks per (tile, src-half)
    n_cores: int      # SPMD width
    layers: int = 2   # debug knob (1 = stop after layer-0 AllGather)

    @property
    def nh(self):
        return self.tph * 128

    @property
    def nt(self):
        return 2 * self.nh

    @property
    def ch_tile(self):       # chunks per dst tile
        return 2 * self.k_fix

    @property
    def chunks(self):        # chunks per core
        return self.tph * self.ch_tile


# ----------------------------------------------------------------- host prep

def _jp(j, nh_real, nh):
    """node id -> padded table row"""
    return (j // nh_real) * nh + (j % nh_real)


def edge_tiles_for_core(edge_index, s, cfg: Cfg):
    """Per dst-tile lists of (srcj, dst_local) with self loops and pad-dst
    dummies, sorted by dst. Returns list over tiles of (lo_src, lo_dl,
    hi_src, hi_dl) arrays (srcj in padded row space)."""
    nh_real, nh = cfg.nh_real, cfg.nh
    src = np.asarray(edge_index[0])
    dst = np.asarray(edge_index[1])
    sel = (dst // nh_real) == s
    srcj = _jp(src[sel], nh_real, nh)
    dstl = _jp(dst[sel], nh_real, nh) - s * nh  # [0, nh)
    # self loops (own real dsts)
    own = np.arange(s * nh_real, (s + 1) * nh_real)
    srcj = np.concatenate([srcj, _jp(own, nh_real, nh)])
    dstl = np.concatenate([dstl, np.arange(nh_real)])
    # pad dst dummies: src = the pad row itself (zero features)
    pad = np.arange(nh_real, nh)
    srcj = np.concatenate([srcj, s * nh + pad])
    dstl = np.concatenate([dstl, pad])
    order = np.argsort(dstl, kind="stable")
    srcj, dstl = srcj[order], dstl[order]
    tiles = []
    bounds = np.searchsorted(dstl, np.arange(0, nh + 128, 128))
    for t in range(cfg.tph):
        a, b = bounds[t], bounds[t + 1]
        ts_, tl = srcj[a:b], dstl[a:b] - t * 128
        lo = ts_ < nh
        tiles.append((ts_[lo], tl[lo], ts_[~lo] - nh, tl[~lo]))
    return tiles


def wrap_idx_call(idx, k_fix):
    """idx list (padded to k_fix*128 with 0) -> wrapped [128, k_fix*8] int16,
    replicated across the 8 16-partition groups."""
    n = k_fix * 128
    full = np.zeros(n, np.int64)
    full[: len(idx)] = idx
    w = np.zeros((16, k_fix * 8), np.int16)
    w[np.arange(n) % 16, np.arange(n) // 16] = full
    return np.tile(w, (8, 1))


def prep_core_edges(edge_index, s, cfg: Cfg):
    tiles = edge_tiles_for_core(edge_index, s, cfg)
    kf, ch, C = cfg.k_fix, cfg.ch_tile, cfg.chunks
    gidx = np.zeros((128, C * 8), np.int16)
    dstl = np.full((128, C), -1.0, np.float32)
    maxk = 0
    for t, (ls, ld, hs, hd) in enumerate(tiles):
        klo, khi = -(-len(ls) // 128), -(-len(hs) // 128)
        maxk = max(maxk, klo, khi)
        if klo > kf or khi > kf:
            raise ValueError(f"k_fix {kf} too small (tile {t}: {klo},{khi})")
        base = t * ch * 8
        gidx[:, base: base + kf * 8] = wrap_idx_call(ls, kf)
        gidx[:, base + kf * 8: base + ch * 8] = wrap_idx_call(hs, kf)
        cb = t * ch
        for arr, off in ((ld, 0), (hd, kf)):
            for i, v in enumerate(arr):
                dstl[i % 128, cb + off + i // 128] = v
    # onehotT fp8 [128, C*128]: block c col e: 1 if dstl[e, c] == d  (lhsT for
    # the ad-expansion matvec).  ohS fp8 [128, C*128]: block c: [e, d] onehot
    # (lhsT for the scatter matmul).
    ohT = np.zeros((128, C * 128), np_f8)
    ohS = np.zeros((128, C * 128), np_f8)
    e_idx, c_idx = np.nonzero(dstl >= 0)  # (partition/edge, chunk)
    d_idx = dstl[e_idx, c_idx].astype(np.int64)
    ohT[d_idx, c_idx * 128 + e_idx] = np_f8(1.0)
    ohS[e_idx, c_idx * 128 + d_idx] = np_f8(1.0)
    return gidx, dstl, ohT, ohS


def required_kfix(edge_index, s, cfg_nokf: Cfg):
    tiles = edge_tiles_for_core(edge_index, s, cfg_nokf)
    mk = 1
    for ls, ld, hs, hd in tiles:
        mk = max(mk, -(-len(ls) // 128), -(-len(hs) // 128))
    return mk


def prep_xT(x, cfg: Cfg):
    """x [n_nodes, 128] -> xT [256, nh] fp32 (half h at rows h*128..)."""
    out = np.zeros((256, cfg.nh), np.float32)
    for h in range(2):
        xs = x[h * cfg.nh_real: (h + 1) * cfg.nh_real]
        out[h * 128: (h + 1) * 128, : xs.shape[0]] = xs.T
    return out


def wrap_apgather_idx(vals, ncols):
    """ap_gather idx layout: [128, ncols] int16, idx i at [i%16, i//16],
    replicated per 16-partition group."""
    w = np.zeros((16, ncols), np.int16)
    for i, v in enumerate(vals):
        w[i % 16, i // 16] = v
    return np.tile(w, (8, 1))


# ------------------------------------------------------------- device module

@with_exitstack
def gat_kernel(ctx: ExitStack, tc: tile.TileContext, io: dict, cfg: Cfg,
               groups):
    nc = tc.nc
    kf, ch, tph = cfg.k_fix, cfg.ch_tile, cfg.tph
    ngrp = cfg.node_grp

    xT_in = io["xT"]
    rhs_node = io["rhs_node"]      # [2, 128, 130]
    b_rep = io["b_rep"]            # [2, 128, 128]
    pw = io["pw"]                  # [128, 1]
    pwA = io["pwA"]                # [128, 1]
    ident = io["ident"]            # [128, 128] f32
    gidx = io["gidx"]              # [128, chunks*8] i16
    ohT_in = io["ohT"]             # [128, chunks*128] f8
    ohS_in = io["ohS"]             # [128, chunks*128] f8
    yout = io["yout"]              # [nh, 128] f32 output

    table = nc.dram_tensor("table", [cfg.nt, 128], f32).ap()
    ccin = nc.dram_tensor("ccin", [128, cfg.nh], f32).ap()
    ccout = nc.dram_tensor("ccout", [256, cfg.nh], f32).ap()

    const = ctx.enter_context(tc.tile_pool(name="const", bufs=1))
    ident_sb = const.tile([128, 128], f32)
    nc.sync.dma_start(ident_sb[:], ident[:])
    pw_sb = const.tile([128, 1], f32)
    nc.sync.dma_start(pw_sb[:], pw[:])
    pwA_sb = const.tile([128, 1], f32)
    nc.sync.dma_start(pwA_sb[:], pwA[:])
    msel0_sb = const.tile([128, 1], f32)
    nc.sync.dma_start(msel0_sb[:], io["msel0"][:])
    msel1_sb = const.tile([128, 1], f32)
    nc.sync.dma_start(msel1_sb[:], io["msel1"][:])
    rhs_sb = []
    brep_sb = []
    for l in range(2):
        r = const.tile([128, 130], f32, tag=f"rhs{l}")
        nc.sync.dma_start(r[:], rhs_node[l])
        rhs_sb.append(r)
        b = const.tile([128, 128], f32, tag=f"brep{l}")
        nc.sync.dma_start(b[:], b_rep[l])
        brep_sb.append(b)
    ad_f32 = const.tile([128, tph], f32)      # own-half ad (masked accumulate)
    ad_own = const.tile([128, tph, 2], bf16)  # (hi, lo) per tile

    for l in range(cfg.layers):
        xsrc = xT_in if l == 0 else ccout

        # ---------------- node phase ----------------
        with tc.tile_pool(name=f"nd{l}", bufs=3) as npool, \
             tc.tile_pool(name=f"ndp{l}", bufs=4, space="PSUM") as nppool:
            for half in range(2):
                for g in range(tph // ngrp):
                    cb = g * ngrp * 128
                    xt = npool.tile([128, ngrp * 128], f32, tag="xt")
                    nc.sync.dma_start(
                        xt[:], xsrc[half * 128:(half + 1) * 128, cb:cb + ngrp * 128])
                    stage = npool.tile([128, ngrp, 130], f32, tag="stage")
                    for i in range(ngrp):
                        ph = nppool.tile([128, 130], f32)
                        nc.tensor.matmul(ph[:], xt[:, i * 128:(i + 1) * 128],
                                         rhs_sb[l][:], start=True, stop=True)
                        nc.scalar.activation(stage[:, i, :], ph[:],
                                             mybir.ActivationFunctionType.Identity)
                    # bit-steal: as fp16 bits -> low bytes of h0, h1
                    asf = npool.tile([128, ngrp], f16, tag="asf")
                    nc.vector.tensor_copy(asf[:], stage[:, :, 128:129])
                    b32 = npool.tile([128, ngrp], i32, tag="b32")
                    nc.vector.tensor_copy(b32[:], asf[:].bitcast(u16))
                    hi8 = npool.tile([128, ngrp], i32, tag="hi8")
                    nc.vector.tensor_scalar(hi8[:], b32[:], 8, None,
                                            mybir.AluOpType.logical_shift_right)
                    lo8 = npool.tile([128, ngrp], i32, tag="lo8")
                    nc.vector.tensor_scalar(lo8[:], b32[:], 0xFF, None,
                                            mybir.AluOpType.bitwise_and)
                    h0 = stage[:, :, 0:1].bitcast(i32)
                    h1 = stage[:, :, 1:2].bitcast(i32)
                    nc.vector.tensor_scalar(h0, h0, -256, None,
                                            mybir.AluOpType.bitwise_and)
                    nc.vector.tensor_tensor(h0, h0, hi8[:],
                                            mybir.AluOpType.bitwise_or)
                    nc.vector.tensor_scalar(h1, h1, -256, None,
                                            mybir.AluOpType.bitwise_and)
                    nc.vector.tensor_tensor(h1, h1, lo8[:],
                                            mybir.AluOpType.bitwise_or)
                    rb = half * cfg.nh + cb
                    tview = table[rb:rb + ngrp * 128, :].rearrange(
                        "(i p) d -> p i d", p=128)
                    nc.sync.dma_start(tview, stage[:, :, 0:128])
                    # ad of own-half dsts, via per-core half mask
                    tb = g * ngrp
                    if half == 0:
                        nc.vector.scalar_tensor_tensor(
                            ad_f32[:, tb:tb + ngrp], stage[:, :, 129:130],
                            msel0_sb[:], stage[:, :, 129:130],
                            mybir.AluOpType.mult, mybir.AluOpType.bypass)
                    else:
                        nc.vector.scalar_tensor_tensor(
                            ad_f32[:, tb:tb + ngrp], stage[:, :, 129:130],
                            msel1_sb[:], ad_f32[:, tb:tb + ngrp],
                            mybir.AluOpType.mult, mybir.AluOpType.add)

        # ad_own hi/lo bf16 pair (fp32-accurate matvec operand)
        nc.vector.tensor_copy(ad_own[:, :, 0:1], ad_f32[:])
        nc.vector.tensor_tensor(ad_own[:, :, 1:2], ad_f32[:],
                                ad_own[:, :, 0:1], mybir.AluOpType.subtract)

        # ---------------- edge phase ----------------
        with tc.tile_pool(name=f"ed{l}", bufs=3) as ep, \
             tc.tile_pool(name=f"eds{l}", bufs=4) as sp, \
             tc.tile_pool(name=f"edp{l}", bufs=2, space="PSUM") as pp, \
             tc.tile_pool(name=f"edpa{l}", bufs=2, space="PSUM") as pa:
            for t in range(tph):
                idx_t = ep.tile([128, ch * 8], i16, tag="idx")
                nc.sync.dma_start(idx_t[:], gidx[:, t * ch * 8:(t + 1) * ch * 8])
                G = ep.tile([128, ch, 128], f32, tag="G")
                nc.gpsimd.dma_gather(G[:, 0:kf, :], table[0:cfg.nh, :],
                                     idx_t[:, 0:kf * 8], kf * 128, kf * 128, 128,
                                     queue_num=0)
                nc.gpsimd.dma_gather(G[:, kf:ch, :], table[cfg.nh:cfg.nt, :],
                                     idx_t[:, kf * 8:ch * 8], kf * 128, kf * 128, 128,
                                     queue_num=0)
                ohT_t = ep.tile([128, ch * 128], f8, tag="ohT")
                nc.sync.dma_start(ohT_t[:], ohT_in[:, t * ch * 128:(t + 1) * ch * 128])
                ohS_t = ep.tile([128, ch * 128], f8, tag="ohS")
                nc.sync.dma_start(ohS_t[:], ohS_in[:, t * ch * 128:(t + 1) * ch * 128])

                adx_ps = pa.tile([128, ch, 2], f32, tag="adx")
                for c in range(ch):
                    nc.tensor.matmul(adx_ps[:, c, :],
                                     ohT_t[:, c * 128:(c + 1) * 128],
                                     ad_own[:, t, :], start=True, stop=True,
                                     skip_group_check=True)
                adh = sp.tile([128, ch], f32, tag="adh")
                nc.scalar.activation(adh[:], adx_ps[:, :, 0:1],
                                     mybir.ActivationFunctionType.Identity)
                adx = sp.tile([128, ch], f32, tag="adx_sb")
                nc.vector.tensor_tensor(adx[:], adh[:], adx_ps[:, :, 1:2],
                                        mybir.AluOpType.add)

                # as extraction from stolen bits
                comb = sp.tile([128, ch], i32, tag="comb")
                nc.vector.tensor_scalar(comb[:], G[:, :, 0:1].bitcast(i32),
                                        0xFF, 8, mybir.AluOpType.bitwise_and,
                                        mybir.AluOpType.logical_shift_left)
                clo = sp.tile([128, ch], i32, tag="clo")
                nc.vector.tensor_scalar(clo[:], G[:, :, 1:2].bitcast(i32),
                                        0xFF, None, mybir.AluOpType.bitwise_and)
                nc.vector.tensor_tensor(comb[:], comb[:], clo[:],
                                        mybir.AluOpType.bitwise_or)
                cu16 = sp.tile([128, ch], u16, tag="cu16")
                nc.vector.tensor_copy(cu16[:], comb[:])
                asx = sp.tile([128, ch], f32, tag="asx")
                nc.vector.tensor_copy(asx[:], cu16[:].bitcast(f16))

                z = sp.tile([128, ch], f32, tag="z")
                nc.vector.tensor_tensor(z[:], asx[:], adx[:], mybir.AluOpType.add)
                r = sp.tile([128, ch], f32, tag="r")
                nc.scalar.activation(r[:], z[:], mybir.ActivationFunctionType.Relu)
                tq = sp.tile([128, ch], f32, tag="tq")
                nc.vector.scalar_tensor_tensor(tq[:], r[:], 1.0 / NEG_SLOPE - 1.0,
                                               z[:], mybir.AluOpType.mult,
                                               mybir.AluOpType.add)
                w = sp.tile([128, ch], f32, tag="w")
                nc.scalar.activation(w[:], tq[:], mybir.ActivationFunctionType.Exp,
                                     scale=NEG_SLOPE)
                whi = sp.tile([128, ch], bf16, tag="whi")
                nc.vector.tensor_copy(whi[:], w[:])
                wlo = sp.tile([128, ch], bf16, tag="wlo")
                nc.vector.tensor_tensor(wlo[:], w[:], whi[:],
                                        mybir.AluOpType.subtract)

                rhs = ep.tile([128, ch, 258], bf16, tag="rhs")
                for c in range(ch):
                    nc.scalar.activation(rhs[:, c, 0:128], G[:, c, 0:128],
                                         mybir.ActivationFunctionType.Identity,
                                         scale=w[:, c:c + 1])
                    nc.vector.scalar_tensor_tensor(
                        rhs[:, c, 128:256], G[:, c, 0:128], w[:, c:c + 1],
                        rhs[:, c, 0:128], mybir.AluOpType.mult,
                        mybir.AluOpType.subtract)
                nc.vector.tensor_copy(rhs[:, :, 256:257], whi[:])
                nc.vector.tensor_copy(rhs[:, :, 257:258], wlo[:])

                out_ps = pp.tile([128, 258], f32, tag="out")
                for c in range(ch):
                    nc.tensor.matmul(out_ps[:], ohS_t[:, c * 128:(c + 1) * 128],
                                     rhs[:, c, :],
                                     start=(c == 0), stop=(c == ch - 1))

                NNh = sp.tile([128, 128], f32, tag="NNh")
                nc.scalar.activation(NNh[:], out_ps[:, 0:128],
                                     mybir.ActivationFunctionType.Identity)
                sh = sp.tile([128, 1], f32, tag="sh")
                nc.scalar.activation(sh[:], out_ps[:, 256:257],
                                     mybir.ActivationFunctionType.Identity)
                NN = sp.tile([128, 128], f32, tag="NN")
                nc.vector.tensor_tensor(NN[:], NNh[:],
                                        out_ps[:, 128:256], mybir.AluOpType.add)
                s2 = sp.tile([128, 1], f32, tag="s2")
                nc.vector.tensor_tensor(s2[:], sh[:],
                                        out_ps[:, 257:258], mybir.AluOpType.add)
                rs = sp.tile([128, 1], f32, tag="rs")
                nc.vector.reciprocal(rs[:], s2[:])
                o2 = sp.tile([128, 128], f32, tag="o2")
                nc.vector.scalar_tensor_tensor(o2[:], NN[:], rs[:], brep_sb[l][:],
                                               mybir.AluOpType.mult,
                                               mybir.AluOpType.add)
                # prelu(x) = relu(x) + pw*(x - relu(x)); 2-input DVE forms only
                # (1-input tensor_scalar would take the 2-port perf-mode lock
                # that GpSimd's SWDGE descriptor generation contends on)
                pr = sp.tile([128, 128], f32, tag="pr")
                nc.scalar.activation(pr[:], o2[:],
                                     mybir.ActivationFunctionType.Relu)
                dneg = sp.tile([128, 128], f32, tag="dneg")
                nc.vector.tensor_tensor(dneg[:], o2[:], pr[:],
                                        mybir.AluOpType.subtract)
                o3 = sp.tile([128, 128], f32, tag="o3")
                nc.vector.scalar_tensor_tensor(o3[:], dneg[:], pw_sb[:], pr[:],
                                               mybir.AluOpType.mult,
                                               mybir.AluOpType.add)
                if l == 0:
                    trp = pp.tile([128, 128], f32, tag="trp")
                    nc.tensor.transpose(trp[:], o3[:], ident_sb[:])
                    trs = sp.tile([128, 128], f32, tag="trs")
                    nc.scalar.activation(trs[:], trp[:],
                                         mybir.ActivationFunctionType.Identity)
                    nc.sync.dma_start(ccin[:, t * 128:(t + 1) * 128], trs[:])
                else:
                    nc.sync.dma_start(yout[t * 128:(t + 1) * 128, :], o3[:])

        if l == 0:
            nc.gpsimd.collective_compute(
                "AllGather", mybir.AluOpType.bypass, replica_groups=groups,
                ins=[ccin.opt()], outs=[ccout.opt()])


# ------------------------------------------------------------------ builder

def build_module(cfg: Cfg):
    from concourse import bacc
    nc = bacc.Bacc("TRN2", target_bir_lowering=False, debug=False,
                   num_devices=cfg.n_cores)
    io = {}

    def inp(name, shape, dt):
        io[name] = nc.dram_tensor(name, shape, dt, kind="ExternalInput").ap()

    inp("xT", [256, cfg.nh], f32)
    inp("rhs_node", [2, 128, 130], f32)
    inp("b_rep", [2, 128, 128], f32)
    inp("pw", [128, 1], f32)
    inp("pwA", [128, 1], f32)
    inp("ident", [128, 128], f32)
    inp("gidx", [128, cfg.chunks * 8], i16)
    inp("ohT", [128, cfg.chunks * 128], f8)
    inp("ohS", [128, cfg.chunks * 128], f8)
    inp("msel0", [128, 1], f32)
    inp("msel1", [128, 1], f32)
    io["yout"] = nc.dram_tensor("yout", [cfg.nh, 128], f32,
                                kind="ExternalOutput").ap()

    pairs = [[i, i + 1] for i in range(0, cfg.n_cores, 2)]
    with tile.TileContext(nc) as tc:
        gat_kernel(tc, io, cfg, pairs)
    nc.compile()
    return nc


def build_in_maps(inputs, cfg: Cfg):
    """Full problem inputs -> per-core in_maps (host preprocessing)."""
    graphs = [inputs["edge_index_g1_pos"], inputs["edge_index_g2_pos"],
              inputs["edge_index_g1_neg"], inputs["edge_index_g2_neg"]]
    params = [("pos" if k < 2 else "neg") for k in range(4)]

    xT = prep_xT(np.asarray(inputs["x"], np.float32), cfg)
    pw_v = float(np.asarray(inputs["prelu_w"]))
    pw = np.full((128, 1), pw_v, np.float32)
    pwA = np.full((128, 1), 1.0 - pw_v, np.float32)
    ident = np.eye(128, dtype=np.float32)

    rhs_by_p, brep_by_p = {}, {}
    for p in ("pos", "neg"):
        W = np.asarray(inputs[f"W_{p}"], np.float32)
        asrc = np.asarray(inputs[f"a_src_{p}"], np.float32)
        adst = np.asarray(inputs[f"a_dst_{p}"], np.float32)
        b = np.asarray(inputs[f"b_{p}"], np.float32)
        rhs = np.zeros((2, 128, 130), np.float32)
        br = np.zeros((2, 128, 128), np.float32)
        for l in range(2):
            rhs[l, :, 0:128] = W[l]
            rhs[l, :, 128] = W[l] @ asrc[l]
            rhs[l, :, 129] = W[l] @ adst[l]
            br[l] = np.tile(b[l], (128, 1))
        rhs_by_p[p] = rhs
        brep_by_p[p] = br

    in_maps = []
    for core in range(cfg.n_cores):
        k, s = core // 2, core % 2
        gidx, dstl, ohT, ohS = prep_core_edges(np.asarray(graphs[k]), s, cfg)
        in_maps.append({
            "xT": xT, "rhs_node": rhs_by_p[params[k]],
            "b_rep": brep_by_p[params[k]], "pw": pw, "pwA": pwA,
            "ident": ident,
            "gidx": gidx, "ohT": ohT, "ohS": ohS,
            "msel0": np.full((128, 1), 1.0 - s, np.float32),
            "msel1": np.full((128, 1), float(s), np.float32),
        })
    return in_maps


def assemble_outputs(results, cfg: Cfg):
    outs = []
    for k in range(4):
        h0 = results[2 * k]["yout"][: cfg.nh_real]
        h1 = results[2 * k + 1]["yout"][: cfg.nh_real]
        outs.append(np.concatenate([h0, h1], axis=0).astype(np.float32))
    return tuple(outs)


# ------------------------------------------------------------------- kernel

_CACHE = {}

FULL_CFG_BASE = dict(nh_real=25000, tph=196, node_grp=7, n_cores=8)


def make_cfg(graphs):
    cfg0 = Cfg(k_fix=0, **FULL_CFG_BASE)
    kf = 1
    for g in graphs:
        for s in range(2):
            kf = max(kf, required_kfix(np.asarray(g), s, cfg0))
    return Cfg(k_fix=kf, **FULL_CFG_BASE)


def kernel(**inputs):
    graphs = [inputs["edge_index_g1_pos"], inputs["edge_index_g2_pos"],
              inputs["edge_index_g1_neg"], inputs["edge_index_g2_neg"]]
    cfg = make_cfg(graphs)

    if cfg not in _CACHE:
        _CACHE[cfg] = build_module(cfg)
    nc = _CACHE[cfg]

    in_maps = build_in_maps(inputs, cfg)
    from concourse.bass_utils import run_bass_kernel_spmd
    res = run_bass_kernel_spmd(nc, in_maps, list(range(cfg.n_cores)))
    return assemble_outputs(res.results, cfg)



# revision 13
# speedup vs baseline: 1.7599x; 1.7599x over previous
"""GAT (2-layer, 4 graphs) Trainium2 Bass kernel.

Problem: nn_GAT_CL_61658550502129. reference.py semantics:
  4 independent 2-layer single-head GATConv chains (PyG-style, self-loops,
  segment softmax over leaky_relu(a_src.h_j + a_dst.h_i)), prelu between/after.

Sharding (8 NeuronCores): 4 chains x 2 cores. Core 2k+s runs chain k
(g1_pos, g2_pos, g1_neg, g2_neg) for destination-node half s. The only
inter-core exchange is a pairwise AllGather of layer-0 output (layer-1 input).

v2 changes vs baseline:
  - self-loop + pad-dst rows removed from the gather (k_fix 8 -> 7, -12.5%
    Pool SWDGE descgen, the dominant engine); the self-loop term is applied
    densely in the epilogue from the tile's own h block + node-phase as/ad.
  - single-precision bf16 scatter rhs (129 cols vs 258): halves the scatter
    matmul, the rhs prep, and drops the w hi/lo split.
  - leaky/prelu via one DVE stt (max(z, a*z)) instead of relu+recombine.
  - rhs-prep split ACT/DVE to balance engines; epilogue reads PSUM directly.
  - bf16 AllGather + bf16 layer-1 node phase (halves collective + xt DMA).
  - gather queue alternation.

Per-core algorithm (per conv):
  node phase: h = x @ W, as = h.a_src, ad = h.a_dst per 128-node tile on PE
    (lhsT = x_tile^T stationary, rhs = [W | W a_src | W a_dst]).
    h rows written to a DRAM table [NT, 128] f32 with as (fp16 bits) hidden in
    the low bytes of h[0], h[1] (bit-steal); as/ad of own-half dsts kept in
    SBUF; wself = exp(leaky(as+ad)) precomputed per layer.
  edge phase (per 128-dst tile): dma_gather of the table rows for the tile's
    real edges (512B/row, int16 idx, lo/hi half tables), per-edge
    w = exp(leaky(as_src + ad_dst)) (ad via tiny PE matvec with fp8 onehotT),
    then a one-hot scatter matmul into PSUM:
      lhsT = onehot(dst_local) fp8, rhs = [w*G bf16 | w bf16]  (129 cols)
    giving numerator and softmax denominator in one accumulation group; the
    epilogue adds the dense self-loop term wself*h_own before normalizing.

kernel(**inputs) -> (e1p, e2p, e1n, e2n), matching reference.reference.
Self-contained: only needs the system bass/concourse toolchain.
"""
import sys

sys.path.insert(0, "/opt/trn_rl_repo")

from contextlib import ExitStack
from dataclasses import dataclass

import ml_dtypes
import numpy as np

import concourse.bass as bass
import concourse.tile as tile
from concourse import mybir
from concourse._compat import with_exitstack

f32 = mybir.dt.float32
f16 = mybir.dt.float16
bf16 = mybir.dt.bfloat16
f8 = mybir.dt.float8e4
i16 = mybir.dt.int16
i32 = mybir.dt.int32
u16 = mybir.dt.uint16

np_bf16 = ml_dtypes.bfloat16
np_f8 = ml_dtypes.float8_e4m3

D = 128
NEG_SLOPE = 0.2


@dataclass(frozen=True)
class Cfg:
    nh_real: int      # real nodes per half
    tph: int          # dst tiles per half (nh_pad = tph*128)
    node_grp: int     # node tiles per node-phase group (divides tph)
    k_fix: int        # gather chunks per (tile, src-half)
    n_cores: int      # SPMD width
    layers: int = 2   # debug knob (1 = stop after layer-0 AllGather)

    @property
    def nh(self):
        return self.tph * 128

    @property
    def nt(self):
        return 2 * self.nh

    @property
    def ch_tile(self):       # chunks per dst tile
        return 2 * self.k_fix

    @property
    def chunks(self):        # chunks per core
        return self.tph * self.ch_tile


# ----------------------------------------------------------------- host prep

def _jp(j, nh_real, nh):
    """node id -> padded table row"""
    return (j // nh_real) * nh + (j % nh_real)


def edge_tiles_for_core(edge_index, s, cfg: Cfg):
    """Per dst-tile lists of (srcj, dst_local) -- REAL edges only (self loops
    handled densely in the epilogue). Returns list over tiles of (lo_src,
    lo_dl, hi_src, hi_dl) arrays (srcj in padded row space)."""
    nh_real, nh = cfg.nh_real, cfg.nh
    src = np.asarray(edge_index[0])
    dst = np.asarray(edge_index[1])
    sel = (dst // nh_real) == s
    srcj = _jp(src[sel], nh_real, nh)
    dstl = _jp(dst[sel], nh_real, nh) - s * nh  # [0, nh)
    order = np.argsort(dstl, kind="stable")
    srcj, dstl = srcj[order], dstl[order]
    tiles = []
    bounds = np.searchsorted(dstl, np.arange(0, nh + 128, 128))
    for t in range(cfg.tph):
        a, b = bounds[t], bounds[t + 1]
        ts_, tl = srcj[a:b], dstl[a:b] - t * 128
        lo = ts_ < nh
        tiles.append((ts_[lo], tl[lo], ts_[~lo] - nh, tl[~lo]))
    return tiles


def wrap_idx_call(idx, k_fix):
    """idx list (padded to k_fix*128 with 0) -> wrapped [128, k_fix*8] int16,
    replicated across the 8 16-partition groups."""
    n = k_fix * 128
    full = np.zeros(n, np.int64)
    full[: len(idx)] = idx
    w = np.zeros((16, k_fix * 8), np.int16)
    w[np.arange(n) % 16, np.arange(n) // 16] = full
    return np.tile(w, (8, 1))


def prep_core_edges(edge_index, s, cfg: Cfg):
    tiles = edge_tiles_for_core(edge_index, s, cfg)
    kf, ch, C = cfg.k_fix, cfg.ch_tile, cfg.chunks
    gidx = np.zeros((128, C * 8), np.int16)
    dstl = np.full((128, C), -1.0, np.float32)
    for t, (ls, ld, hs, hd) in enumerate(tiles):
        klo, khi = -(-len(ls) // 128), -(-len(hs) // 128)
        if klo > kf or khi > kf:
            raise ValueError(f"k_fix {kf} too small (tile {t}: {klo},{khi})")
        base = t * ch * 8
        gidx[:, base: base + kf * 8] = wrap_idx_call(ls, kf)
        gidx[:, base + kf * 8: base + ch * 8] = wrap_idx_call(hs, kf)
        cb = t * ch
        for arr, off in ((ld, 0), (hd, kf)):
            for i, v in enumerate(arr):
                dstl[i % 128, cb + off + i // 128] = v
    # onehotT fp8 [128, C*128]: block c col e: 1 if dstl[e, c] == d  (lhsT for
    # the ad-expansion matvec).  ohS fp8 [128, C*128]: block c: [e, d] onehot
    # (lhsT for the scatter matmul).
    ohT = np.zeros((128, C * 128), np_f8)
    ohS = np.zeros((128, C * 128), np_f8)
    e_idx, c_idx = np.nonzero(dstl >= 0)  # (partition/edge, chunk)
    d_idx = dstl[e_idx, c_idx].astype(np.int64)
    ohT[d_idx, c_idx * 128 + e_idx] = np_f8(1.0)
    ohS[e_idx, c_idx * 128 + d_idx] = np_f8(1.0)
    return gidx, dstl, ohT, ohS


def required_kfix(edge_index, s, cfg_nokf: Cfg):
    tiles = edge_tiles_for_core(edge_index, s, cfg_nokf)
    mk = 1
    for ls, ld, hs, hd in tiles:
        mk = max(mk, -(-len(ls) // 128), -(-len(hs) // 128))
    return mk


def prep_xT(x, cfg: Cfg):
    """x [n_nodes, 128] -> xT [256, nh] fp32 (half h at rows h*128..)."""
    out = np.zeros((256, cfg.nh), np.float32)
    for h in range(2):
        xs = x[h * cfg.nh_real: (h + 1) * cfg.nh_real]
        out[h * 128: (h + 1) * 128, : xs.shape[0]] = xs.T
    return out


# ------------------------------------------------------------- device module

@with_exitstack
def gat_kernel(ctx: ExitStack, tc: tile.TileContext, io: dict, cfg: Cfg,
               groups):
    nc = tc.nc
    kf, ch, tph = cfg.k_fix, cfg.ch_tile, cfg.tph
    ngrp = cfg.node_grp

    xT_in = io["xT"]
    rhs_node = io["rhs_node"]      # [2, 128, 130] f32 (layer 0)
    rhs_nodeb = io["rhs_nodeb"]    # [2, 128, 130] bf16 (layer 1)
    b_rep = io["b_rep"]            # [2, 128, 128]
    pw = io["pw"]                  # [128, 1]
    c02 = io["c02"]                # [128, 1] = NEG_SLOPE
    zc = io["zc"]                  # [128, 128] f32 zeros
    ident = io["ident"]            # [128, 128] f32
    gidx = io["gidx"]              # [128, chunks*8] i16
    ohT_in = io["ohT"]             # [128, chunks*128] f8
    ohS_in = io["ohS"]             # [128, chunks*128] f8
    yout = io["yout"]              # [nh, 128] f32 output

    table = nc.dram_tensor("table", [cfg.nt, 128], f32).ap()
    # layer-0 -> layer-1 exchange, split in two so the first AllGather
    # overlaps the second half of the layer-0 edge phase. The split must
    # align to node-phase group granularity (ngrp tiles).
    tphA = (tph // (2 * ngrp)) * ngrp
    nhA = tphA * 128
    if tphA > 0:
        ccinA = nc.dram_tensor("ccinA", [128, nhA], bf16).ap()
        ccoutA = nc.dram_tensor("ccoutA", [256, nhA], bf16).ap()
    else:
        ccinA = ccoutA = None
    ccinB = nc.dram_tensor("ccinB", [128, cfg.nh - nhA], bf16).ap()
    ccoutB = nc.dram_tensor("ccoutB", [256, cfg.nh - nhA], bf16).ap()

    const = ctx.enter_context(tc.tile_pool(name="const", bufs=1))
    ident_sb = const.tile([128, 128], f32)
    nc.sync.dma_start(ident_sb[:], ident[:])
    pw_sb = const.tile([128, 1], f32)
    nc.sync.dma_start(pw_sb[:], pw[:])
    c02_sb = const.tile([128, 1], f32)
    nc.sync.dma_start(c02_sb[:], c02[:])
    zc_sb = const.tile([128, 128], f32)
    nc.sync.dma_start(zc_sb[:], zc[:])
    msel0_sb = const.tile([128, 1], f32)
    nc.sync.dma_start(msel0_sb[:], io["msel0"][:])
    msel1_sb = const.tile([128, 1], f32)
    nc.sync.dma_start(msel1_sb[:], io["msel1"][:])
    rhs_sb = []
    brep_sb = []
    for l in range(2):
        if l == 0:
            r = const.tile([128, 130], f32, tag=f"rhs{l}")
            nc.sync.dma_start(r[:], rhs_node[l])
        else:
            r = const.tile([128, 130], bf16, tag=f"rhs{l}")
            nc.sync.dma_start(r[:], rhs_nodeb[l])
        rhs_sb.append(r)
        b = const.tile([128, 128], f32, tag=f"brep{l}")
        nc.sync.dma_start(b[:], b_rep[l])
        brep_sb.append(b)
    ad_f32 = const.tile([128, tph], f32)      # own-half ad (masked accumulate)
    as_f32 = const.tile([128, tph], f32)      # own-half as
    ad_own = const.tile([128, tph, 2], bf16)  # (hi, lo) per tile
    wself = const.tile([128, tph], f32)       # exp(leaky(as+ad)) per own dst
    hself = const.tile([128, tph, 128], bf16)  # own-half h rows (self term)

    for l in range(cfg.layers):
        # ---------------- node phase ----------------
        with tc.tile_pool(name=f"nd{l}", bufs=3) as npool, \
             tc.tile_pool(name=f"ndp{l}", bufs=4, space="PSUM") as nppool:
            for half in range(2):
                for g in range(tph // ngrp):
                    cb = g * ngrp * 128
                    xt = npool.tile([128, ngrp * 128], f32 if l == 0 else bf16,
                                    tag="xt")
                    rsl = slice(half * 128, (half + 1) * 128)
                    if l == 0:
                        xv = xT_in[rsl, cb:cb + ngrp * 128]
                    elif cb < nhA:
                        xv = ccoutA[rsl, cb:cb + ngrp * 128]
                    else:
                        xv = ccoutB[rsl, cb - nhA:cb - nhA + ngrp * 128]
                    nc.sync.dma_start(xt[:], xv)
                    stage = npool.tile([128, ngrp, 130], f32, tag="stage")
                    for i in range(ngrp):
                        ph = nppool.tile([128, 130], f32)
                        nc.tensor.matmul(ph[:], xt[:, i * 128:(i + 1) * 128],
                                         rhs_sb[l][:], start=True, stop=True)
                        nc.scalar.activation(stage[:, i, :], ph[:],
                                             mybir.ActivationFunctionType.Identity)
                    # bit-steal: as fp16 bits -> low bytes of h0, h1
                    asf = npool.tile([128, ngrp], f16, tag="asf")
                    nc.vector.tensor_copy(asf[:], stage[:, :, 128:129])
                    b32 = npool.tile([128, ngrp], i32, tag="b32")
                    nc.vector.tensor_copy(b32[:], asf[:].bitcast(u16))
                    hi8 = npool.tile([128, ngrp], i32, tag="hi8")
                    nc.vector.tensor_scalar(hi8[:], b32[:], 8, None,
                                            mybir.AluOpType.logical_shift_right)
                    lo8 = npool.tile([128, ngrp], i32, tag="lo8")
                    nc.vector.tensor_scalar(lo8[:], b32[:], 0xFF, None,
                                            mybir.AluOpType.bitwise_and)
                    h0 = stage[:, :, 0:1].bitcast(i32)
                    h1 = stage[:, :, 1:2].bitcast(i32)
                    nc.vector.tensor_scalar(h0, h0, -256, None,
                                            mybir.AluOpType.bitwise_and)
                    nc.vector.tensor_tensor(h0, h0, hi8[:],
                                            mybir.AluOpType.bitwise_or)
                    nc.vector.tensor_scalar(h1, h1, -256, None,
                                            mybir.AluOpType.bitwise_and)
                    nc.vector.tensor_tensor(h1, h1, lo8[:],
                                            mybir.AluOpType.bitwise_or)
                    rb = half * cfg.nh + cb
                    tview = table[rb:rb + ngrp * 128, :].rearrange(
                        "(i p) d -> p i d", p=128)
                    nc.sync.dma_start(tview, stage[:, :, 0:128])
                    # as/ad/h of own-half dsts, via per-core half mask
                    tb = g * ngrp
                    for dst_t, col in ((as_f32, 128), (ad_f32, 129)):
                        if half == 0:
                            nc.vector.scalar_tensor_tensor(
                                dst_t[:, tb:tb + ngrp], stage[:, :, col:col + 1],
                                msel0_sb[:], stage[:, :, col:col + 1],
                                mybir.AluOpType.mult, mybir.AluOpType.bypass)
                        else:
                            nc.vector.scalar_tensor_tensor(
                                dst_t[:, tb:tb + ngrp], stage[:, :, col:col + 1],
                                msel1_sb[:], dst_t[:, tb:tb + ngrp],
                                mybir.AluOpType.mult, mybir.AluOpType.add)
                    if half == 0:
                        nc.vector.scalar_tensor_tensor(
                            hself[:, tb:tb + ngrp, :], stage[:, :, 0:128],
                            msel0_sb[:], stage[:, :, 0:128],
                            mybir.AluOpType.mult, mybir.AluOpType.bypass)
                    else:
                        nc.vector.scalar_tensor_tensor(
                            hself[:, tb:tb + ngrp, :], stage[:, :, 0:128],
                            msel1_sb[:], hself[:, tb:tb + ngrp, :],
                            mybir.AluOpType.mult, mybir.AluOpType.add)

        # ad_own hi/lo bf16 pair (fp32-accurate matvec operand)
        nc.vector.tensor_copy(ad_own[:, :, 0:1], ad_f32[:])
        nc.vector.tensor_tensor(ad_own[:, :, 1:2], ad_f32[:],
                                ad_own[:, :, 0:1], mybir.AluOpType.subtract)
        # wself = exp(leaky(as + ad)) for own dsts (pad rows -> exp(0) = 1,
        # which keeps the pad-row denominator nonzero)
        zo = const.tile([128, tph], f32, tag=f"zo{l}")
        nc.vector.tensor_tensor(zo[:], as_f32[:], ad_f32[:],
                                mybir.AluOpType.add)
        nc.vector.scalar_tensor_tensor(zo[:], zo[:], c02_sb[:], zo[:],
                                       mybir.AluOpType.mult,
                                       mybir.AluOpType.max)
        nc.scalar.activation(wself[:], zo[:],
                             mybir.ActivationFunctionType.Exp)

        # ---------------- edge phase ----------------
        with tc.tile_pool(name=f"ed{l}", bufs=3) as ep, \
             tc.tile_pool(name=f"eds{l}", bufs=4) as sp, \
             tc.tile_pool(name=f"edp{l}", bufs=2, space="PSUM") as pp, \
             tc.tile_pool(name=f"edpa{l}", bufs=2, space="PSUM") as pa:
            for t in range(tph):
                idx_t = ep.tile([128, ch * 8], i16, tag="idx")
                nc.sync.dma_start(idx_t[:], gidx[:, t * ch * 8:(t + 1) * ch * 8])
                G = ep.tile([128, ch, 128], f32, tag="G")
                nc.gpsimd.dma_gather(G[:, 0:kf, :], table[0:cfg.nh, :],
                                     idx_t[:, 0:kf * 8], kf * 128, kf * 128, 128,
                                     queue_num=0)
                nc.gpsimd.dma_gather(G[:, kf:ch, :], table[cfg.nh:cfg.nt, :],
                                     idx_t[:, kf * 8:ch * 8], kf * 128, kf * 128, 128,
                                     queue_num=0)
                ohT_t = ep.tile([128, ch * 128], f8, tag="ohT")
                nc.sync.dma_start(ohT_t[:], ohT_in[:, t * ch * 128:(t + 1) * ch * 128])
                ohS_t = ep.tile([128, ch * 128], f8, tag="ohS")
                nc.sync.dma_start(ohS_t[:], ohS_in[:, t * ch * 128:(t + 1) * ch * 128])

                adx_ps = pa.tile([128, ch, 2], f32, tag="adx")
                for c in range(ch):
                    nc.tensor.matmul(adx_ps[:, c, :],
                                     ohT_t[:, c * 128:(c + 1) * 128],
                                     ad_own[:, t, :], start=True, stop=True,
                                     skip_group_check=True)

                # as extraction from stolen bits
                comb = sp.tile([128, ch], i32, tag="comb")
                nc.vector.tensor_scalar(comb[:], G[:, :, 0:1].bitcast(i32),
                                        0xFF, 8, mybir.AluOpType.bitwise_and,
                                        mybir.AluOpType.logical_shift_left)
                clo = sp.tile([128, ch], i32, tag="clo")
                nc.vector.tensor_scalar(clo[:], G[:, :, 1:2].bitcast(i32),
                                        0xFF, None, mybir.AluOpType.bitwise_and)
                nc.vector.tensor_tensor(comb[:], comb[:], clo[:],
                                        mybir.AluOpType.bitwise_or)
                cu16 = sp.tile([128, ch], u16, tag="cu16")
                nc.vector.tensor_copy(cu16[:], comb[:])
                asx = sp.tile([128, ch], f32, tag="asx")
                nc.vector.tensor_copy(asx[:], cu16[:].bitcast(f16))

                # z = asx + ad, leaky via max(z, 0.2 z), w = exp
                # (one PSUM operand per DVE op)
                z = sp.tile([128, ch], f32, tag="z")
                nc.vector.tensor_tensor(z[:], adx_ps[:, :, 0:1], asx[:],
                                        mybir.AluOpType.add)
                nc.vector.tensor_tensor(z[:], z[:], adx_ps[:, :, 1:2],
                                        mybir.AluOpType.add)
                tq = sp.tile([128, ch], f32, tag="tq")
                nc.vector.scalar_tensor_tensor(tq[:], z[:], c02_sb[:], z[:],
                                               mybir.AluOpType.mult,
                                               mybir.AluOpType.max)
                w = sp.tile([128, ch], f32, tag="w")
                nc.scalar.activation(w[:], tq[:], mybir.ActivationFunctionType.Exp)

                rhs = ep.tile([128, ch, 129], bf16, tag="rhs")
                for c in range(ch):
                    if c % 3 == 0:
                        nc.scalar.activation(rhs[:, c, 0:128], G[:, c, 0:128],
                                             mybir.ActivationFunctionType.Identity,
                                             scale=w[:, c:c + 1])
                    else:
                        nc.vector.scalar_tensor_tensor(
                            rhs[:, c, 0:128], G[:, c, 0:128], w[:, c:c + 1],
                            zc_sb[:], mybir.AluOpType.mult,
                            mybir.AluOpType.add)
                nc.vector.tensor_copy(rhs[:, :, 128:129], w[:])

                out_ps = pp.tile([128, 129], f32, tag="out")
                for c in range(ch):
                    nc.tensor.matmul(out_ps[:], ohS_t[:, c * 128:(c + 1) * 128],
                                     rhs[:, c, :],
                                     start=(c == 0), stop=(c == ch - 1))

                # epilogue: add dense self-loop term, normalize, bias, prelu
                sp1 = sp.tile([128, 1], f32, tag="sp1")
                nc.vector.tensor_tensor(sp1[:], out_ps[:, 128:129],
                                        wself[:, t:t + 1], mybir.AluOpType.add)
                rs = sp.tile([128, 1], f32, tag="rs")
                nc.vector.reciprocal(rs[:], sp1[:])
                t1 = sp.tile([128, 128], f32, tag="t1")
                nc.vector.scalar_tensor_tensor(t1[:], hself[:, t, :],
                                               wself[:, t:t + 1],
                                               out_ps[:, 0:128],
                                               mybir.AluOpType.mult,
                                               mybir.AluOpType.add)
                o2 = sp.tile([128, 128], f32, tag="o2")
                nc.vector.scalar_tensor_tensor(o2[:], t1[:], rs[:], brep_sb[l][:],
                                               mybir.AluOpType.mult,
                                               mybir.AluOpType.add)
                # prelu(x) = max(x, pw*x) for 0 < pw < 1
                o3 = sp.tile([128, 128], f32, tag="o3")
                nc.vector.scalar_tensor_tensor(o3[:], o2[:], pw_sb[:], o2[:],
                                               mybir.AluOpType.mult,
                                               mybir.AluOpType.max)
                if l == 0:
                    trp = pp.tile([128, 128], f32, tag="trp")
                    nc.tensor.transpose(trp[:], o3[:], ident_sb[:])
                    trs = sp.tile([128, 128], bf16, tag="trs")
                    nc.vector.tensor_copy(trs[:], trp[:])
                    if t < tphA:
                        nc.sync.dma_start(ccinA[:, t * 128:(t + 1) * 128],
                                          trs[:])
                    else:
                        tb_ = (t - tphA) * 128
                        nc.sync.dma_start(ccinB[:, tb_:tb_ + 128], trs[:])
                    if t == tphA - 1:
                        # first-half AllGather overlaps the remaining tiles
                        nc.gpsimd.collective_compute(
                            "AllGather", mybir.AluOpType.bypass,
                            replica_groups=groups,
                            ins=[ccinA.opt()], outs=[ccoutA.opt()])
                else:
                    nc.sync.dma_start(yout[t * 128:(t + 1) * 128, :], o3[:])

        if l == 0:
            nc.gpsimd.collective_compute(
                "AllGather", mybir.AluOpType.bypass, replica_groups=groups,
                ins=[ccinB.opt()], outs=[ccoutB.opt()])


# ------------------------------------------------------------------ builder

def build_module(cfg: Cfg, own_half: int | None = None):
    """own_half: which dst half this core owns. Must be identical across the
    SPMD program, so it is passed per-core via the own_base input instead."""
    from concourse import bacc
    nc = bacc.Bacc("TRN2", target_bir_lowering=False, debug=False,
                   num_devices=cfg.n_cores)
    io = {}

    def inp(name, shape, dt):
        io[name] = nc.dram_tensor(name, shape, dt, kind="ExternalInput").ap()

    inp("xT", [256, cfg.nh], f32)
    inp("rhs_node", [2, 128, 130], f32)
    inp("rhs_nodeb", [2, 128, 130], bf16)
    inp("b_rep", [2, 128, 128], f32)
    inp("pw", [128, 1], f32)
    inp("c02", [128, 1], f32)
    inp("zc", [128, 128], f32)
    inp("ident", [128, 128], f32)
    inp("gidx", [128, cfg.chunks * 8], i16)
    inp("ohT", [128, cfg.chunks * 128], f8)
    inp("ohS", [128, cfg.chunks * 128], f8)
    inp("msel0", [128, 1], f32)
    inp("msel1", [128, 1], f32)
    io["yout"] = nc.dram_tensor("yout", [cfg.nh, 128], f32,
                                kind="ExternalOutput").ap()

    pairs = [[i, i + 1] for i in range(0, cfg.n_cores, 2)]
    with tile.TileContext(nc) as tc:
        gat_kernel(tc, io, cfg, pairs)
    nc.compile()
    return nc


def build_in_maps(inputs, cfg: Cfg):
    """Full problem inputs -> per-core in_maps (host preprocessing)."""
    graphs = [inputs["edge_index_g1_pos"], inputs["edge_index_g2_pos"],
              inputs["edge_index_g1_neg"], inputs["edge_index_g2_neg"]]
    params = [("pos" if k < 2 else "neg") for k in range(4)]

    xT = prep_xT(np.asarray(inputs["x"], np.float32), cfg)
    pw_v = float(np.asarray(inputs["prelu_w"]))
    pw = np.full((128, 1), pw_v, np.float32)
    c02 = np.full((128, 1), NEG_SLOPE, np.float32)
    zc = np.zeros((128, 128), np.float32)
    ident = np.eye(128, dtype=np.float32)

    rhs_by_p, brep_by_p = {}, {}
    for p in ("pos", "neg"):
        W = np.asarray(inputs[f"W_{p}"], np.float32)
        asrc = np.asarray(inputs[f"a_src_{p}"], np.float32)
        adst = np.asarray(inputs[f"a_dst_{p}"], np.float32)
        b = np.asarray(inputs[f"b_{p}"], np.float32)
        rhs = np.zeros((2, 128, 130), np.float32)
        br = np.zeros((2, 128, 128), np.float32)
        for l in range(2):
            rhs[l, :, 0:128] = W[l]
            rhs[l, :, 128] = W[l] @ asrc[l]
            rhs[l, :, 129] = W[l] @ adst[l]
            br[l] = np.tile(b[l], (128, 1))
        rhs_by_p[p] = rhs
        brep_by_p[p] = br

    in_maps = []
    for core in range(cfg.n_cores):
        k, s = core // 2, core % 2
        gidx, dstl, ohT, ohS = prep_core_edges(np.asarray(graphs[k]), s, cfg)
        in_maps.append({
            "xT": xT, "rhs_node": rhs_by_p[params[k]],
            "rhs_nodeb": rhs_by_p[params[k]].astype(np_bf16),
            "b_rep": brep_by_p[params[k]], "pw": pw, "c02": c02, "zc": zc,
            "ident": ident,
            "gidx": gidx, "ohT": ohT, "ohS": ohS,
            "msel0": np.full((128, 1), 1.0 - s, np.float32),
            "msel1": np.full((128, 1), float(s), np.float32),
        })
    return in_maps


def assemble_outputs(results, cfg: Cfg):
    outs = []
    for k in range(4):
        h0 = results[2 * k]["yout"][: cfg.nh_real]
        h1 = results[2 * k + 1]["yout"][: cfg.nh_real]
        outs.append(np.concatenate([h0, h1], axis=0).astype(np.float32))
    return tuple(outs)


# ------------------------------------------------------------------- kernel

_CACHE = {}

FULL_CFG_BASE = dict(nh_real=25000, tph=196, node_grp=7, n_cores=8)


def make_cfg(graphs):
    cfg0 = Cfg(k_fix=0, **FULL_CFG_BASE)
    kf = 1
    for g in graphs:
        for s in range(2):
            kf = max(kf, required_kfix(np.asarray(g), s, cfg0))
    return Cfg(k_fix=kf, **FULL_CFG_BASE)


def kernel(**inputs):
    graphs = [inputs["edge_index_g1_pos"], inputs["edge_index_g2_pos"],
              inputs["edge_index_g1_neg"], inputs["edge_index_g2_neg"]]
    cfg = make_cfg(graphs)

    if cfg not in _CACHE:
        _CACHE[cfg] = build_module(cfg)
    nc = _CACHE[cfg]

    in_maps = build_in_maps(inputs, cfg)
    from concourse.bass_utils import run_bass_kernel_spmd
    res = run_bass_kernel_spmd(nc, in_maps, list(range(cfg.n_cores)))
    return assemble_outputs(res.results, cfg)
